# revision 27
# baseline (speedup 1.0000x reference)
"""Trainium2 Bass kernel for nn_B2BConv1d (Hyena-style back-to-back causal
depthwise convs with gating).

Reference computation (B=2, D=4096, L=2048, channels of x are 3*D interleaved
as c = 3*g + p for stream p in {x1, x2, v}):
    features = causal_dw_conv1d(x, w_proj)          # K=3, per-channel weights
    x1, x2, v = de-interleave(features)             # [B, D, L] each
    z = x2 * v
    z = causal_dw_conv1d(z, repeat(w_short, 16))    # K=7, filter shared per 16ch
    out = x1 * z

Sharding: channels (g in [0, 4096)) split across 8 cores, 512 output channels
per core.  No halo needed (convs are along L, fully local per channel).
The host de-interleaves the 3 streams, casts them to fp16 (halves HBM read
traffic: 12 MiB/core instead of 24 MiB), and precomputes weight tables.

Engine plan per 128-channel x 2048 unit (variant "pc"):
  - TensorE: pf2 = diag-matmul conv3(x2) [3 taps] + pz = conv7(z) [7 taps]
    (10 PE taps instead of the baseline's 13).
  - ScalarE (ACT): tap-2 scaled copies for fv/f1, evacuates pf2/pz PSUM->fp16.
  - VectorE (DVE): PAIR_MAC custom op (2 conv taps in one pass via two
    shifted views) for fv and f1 taps 0-1; tap-2 merge adds and both gate
    multiplies as fp16 2x tensor_tensor.
  - Pool (gpsimd): pad memsets only (walrus rejects TensorScalarPtr on Pool;
    gpsimd tensor_add measured slower than the DVE 2x path).
  - DMA: all loads/stores HWDGE fp16.
Variant "v16" keeps the baseline 13-tap PE structure with fp16 loads.
"""

import numpy as np
from contextlib import ExitStack

B, D, L = 2, 4096, 2048
NCORES = 8
DG = D // NCORES          # 512 output channels per core
CPT = 128                 # channels per partition tile
NT = DG // CPT            # 4 partition tiles per core
K3, K7 = 3, 7
NB = 4                    # PSUM bank tiles per unit
BW = L // NB              # 512 columns per bank tile

import os as _os
DEFAULT_VARIANT = _os.environ.get("KVAR", "pc")

_PROG_CACHE = {}
_PAIR_MAC = None


def _get_pair_mac():
    """Register (once) and return the PAIR_MAC custom DVE op:
    out = in0*s0 + in1*s1 with per-partition scalars s0, s1 — two conv taps
    (two shifted views of the same SBUF tile) in one DVE pass."""
    global _PAIR_MAC
    if _PAIR_MAC is not None:
        return _PAIR_MAC
    import concourse.dve_ops as dve_ops
    from concourse.dve_spec import Spec, Src0, Src1, C0, C1
    from concourse.dve_uop import DveOpSpec
    from concourse.dve_spec import lower

    name = "PAIR_MAC_ANT"
    spec = Spec(
        body=Src0 * C0 + Src1 * C1,
        reference=lambda in0, in1, s0, s1, imm2: (
            in0.astype(np.float32) * s0 + in1.astype(np.float32) * s1),
    )
    # compute the uops sha for this arch so DveOp.compile's pin check passes
    shas = {}
    for ver in ("v3", "v4"):
        opcode = max(dve_ops._SUB_OPCODE_FOR_NAME.values()) + 1
        s = DveOpSpec(name=name, opcode=opcode, uops=lower(spec, ver=ver),
                      rd1_en=True)
        shas[ver] = s.sha(ver)
    op = dve_ops.DveOp(name, spec, subdim=False, uops_sha=shas)
    if name not in dve_ops._SUB_OPCODE_FOR_NAME:
        dve_ops.OPS.append(op)
        dve_ops._SUB_OPCODE_FOR_NAME[name] = (
            max(dve_ops._SUB_OPCODE_FOR_NAME.values()) + 1)
        dve_ops.CUSTOM_DVE_SPECS[name] = spec
    _PAIR_MAC = op
    return op


def build_program(niter=1, variant=DEFAULT_VARIANT, hwloop=False):
    """Build + compile the (SPMD, per-core) Bass program. Same program runs on
    all 8 cores; only the DRAM input contents differ.

    variants:
      "v16"   - fp16 loads, baseline engine split (PE does f2/fv conv3+conv7)
      "pc"    - fp16 loads, PE 10 taps, DVE pair-MACs, Pool merges
      "dma16" - loads + store only (DMA roofline probe)
    """
    import concourse.bacc as bacc
    import concourse.mybir as mybir
    import concourse.tile as tile

    f32 = mybir.dt.float32
    f16 = mybir.dt.float16
    mult = mybir.AluOpType.mult
    add = mybir.AluOpType.add
    Copy = mybir.ActivationFunctionType.Copy

    if variant in ("pc", "pcq", "pcb", "pcw"):
        pair_mac = _get_pair_mac()

    nc = bacc.Bacc("TRN2", target_bir_lowering=False, debug=False)

    x1d = nc.dram_tensor("x1", [B, DG, L], f16, kind="ExternalInput")
    x2d = nc.dram_tensor("x2", [B, DG, L], f16, kind="ExternalInput")
    xvd = nc.dram_tensor("xv", [B, DG, L], f16, kind="ExternalInput")
    w1d = nc.dram_tensor("w1", [DG, K3], f32, kind="ExternalInput")
    w2d = nc.dram_tensor("w2", [DG, K3], f32, kind="ExternalInput")
    wvd = nc.dram_tensor("wv", [DG, K3], f32, kind="ExternalInput")
    d2d = nc.dram_tensor("d2", [CPT, NT * K3 * CPT], f16, kind="ExternalInput")
    dvd = nc.dram_tensor("dv", [CPT, NT * K3 * CPT], f16, kind="ExternalInput")
    d7d = nc.dram_tensor("d7", [CPT, NT * K7 * CPT], f16, kind="ExternalInput")
    outd = nc.dram_tensor("out", [B, DG, L], f16, kind="ExternalOutput")

    with tile.TileContext(nc) as tc:
        with ExitStack() as ctx:
            wpool = ctx.enter_context(tc.tile_pool(name="wpool", bufs=1))
            nb = 3 if variant == "pcb" else 2
            xpool = ctx.enter_context(tc.tile_pool(name="xpool", bufs=nb))
            mpool = ctx.enter_context(tc.tile_pool(name="mpool", bufs=nb))
            opool = ctx.enter_context(tc.tile_pool(name="opool", bufs=2))
            ppool = ctx.enter_context(
                tc.tile_pool(name="ppool", bufs=2, space="PSUM"))
            ppool3 = ctx.enter_context(
                tc.tile_pool(name="ppool3", bufs=2, space="PSUM"))

            # per-partition tap weights, one [CPT, K3] block per g-tile.
            # (Load only what the variant uses: pc/pcq has no fv/f2 stt or
            # dv-diag matmuls, so dvs/w2s stay unloaded.)
            w1s = wpool.tile([CPT, NT * K3], f32)
            wvs = wpool.tile([CPT, NT * K3], f32)
            for gt in range(NT):
                cs = slice(gt * CPT, (gt + 1) * CPT)
                nc.sync.dma_start(w1s[:, gt * K3:(gt + 1) * K3], w1d[cs, :])
                nc.sync.dma_start(wvs[:, gt * K3:(gt + 1) * K3], wvd[cs, :])
            # diag lhsT weight matrices for the PE convs
            d2s = wpool.tile([CPT, NT * K3 * CPT], f16)
            d7s = wpool.tile([CPT, NT * K7 * CPT], f16)
            nc.sync.dma_start(d2s[:], d2d[:, :])
            nc.sync.dma_start(d7s[:], d7d[:, :])
            if variant not in ("pc", "pcq", "pcb", "pcw"):
                dvs = wpool.tile([CPT, NT * K3 * CPT], f16)
                nc.sync.dma_start(dvs[:], dvd[:, :])

            def lhsT(dtile, gt, K, k):
                o = (gt * K + k) * CPT
                return dtile[:, o:o + CPT]

            def load_unit(b, gt):
                cs = slice(gt * CPT, (gt + 1) * CPT)
                xt1 = xpool.tile([CPT, 2 + L], f16, tag="xt1")
                xt2 = xpool.tile([CPT, 2 + L], f16, tag="xt2")
                xtv = xpool.tile([CPT, 2 + L], f16, tag="xtv")
                nc.gpsimd.memset(xt1[:, 0:2], 0.0)
                nc.gpsimd.memset(xt2[:, 0:2], 0.0)
                nc.gpsimd.memset(xtv[:, 0:2], 0.0)
                if variant == "pcq":
                    # spread the three stream loads across three HWDGE
                    # queues (SP / ACT / DVE) for DMA-engine parallelism
                    nc.sync.dma_start(xt1[:, 2:2 + L], x1d[b, cs, :])
                    nc.scalar.dma_start(xt2[:, 2:2 + L], x2d[b, cs, :])
                    nc.vector.dma_start(xtv[:, 2:2 + L], xvd[b, cs, :])
                else:
                    nc.sync.dma_start(xt1[:, 2:2 + L], x1d[b, cs, :])
                    nc.sync.dma_start(xt2[:, 2:2 + L], x2d[b, cs, :])
                    nc.sync.dma_start(xtv[:, 2:2 + L], xvd[b, cs, :])
                return xt1, xt2, xtv

            def one_pass_v16():
                # baseline engine split, fp16 loads. f1 accumulated fp32.
                for b in range(B):
                    for gt in range(NT):
                        cs = slice(gt * CPT, (gt + 1) * CPT)
                        xt1, xt2, xtv = load_unit(b, gt)
                        if variant == "dma16":
                            res = opool.tile([CPT, L], f16, tag="res")
                            nc.scalar.activation(res[:], xt1[:, 2:2 + L], Copy)
                            nc.sync.dma_start(outd[b, cs, :], res[:])
                            continue

                        # f1 path fp32: ACT tap0, DVE taps 1-2.
                        f1 = mpool.tile([CPT, L], f32, tag="f1")
                        nc.scalar.activation(
                            f1[:], xt1[:, 0:L], Copy,
                            scale=w1s[:, gt * K3:gt * K3 + 1])
                        for k in (1, 2):
                            nc.vector.scalar_tensor_tensor(
                                f1[:], xt1[:, k:k + L],
                                w1s[:, gt * K3 + k:gt * K3 + k + 1], f1[:],
                                mult, add)

                        z0 = mpool.tile([CPT, 6 + L], f16, tag="z0")
                        nc.gpsimd.memset(z0[:, 0:6], 0.0)
                        res = opool.tile([CPT, L], f16, tag="res")

                        # software-pipeline by one bank tile
                        pf = {}

                        def conv3s(t):
                            c0 = t * BW
                            pf2 = ppool3.tile([CPT, BW], f32, tag="pf2")
                            pfv = ppool3.tile([CPT, BW], f32, tag="pfv")
                            for k in range(K3):
                                nc.tensor.matmul(
                                    pfv[:], lhsT(dvs, gt, K3, k),
                                    xtv[:, c0 + k:c0 + k + BW],
                                    start=(k == 0), stop=(k == K3 - 1))
                            for k in range(K3):
                                nc.tensor.matmul(
                                    pf2[:], lhsT(d2s, gt, K3, k),
                                    xt2[:, c0 + k:c0 + k + BW],
                                    start=(k == 0), stop=(k == K3 - 1))
                            pf[t] = (pf2, pfv)

                        def zstage(t):
                            c0 = t * BW
                            pf2, pfv = pf.pop(t)
                            fvs = mpool.tile([CPT, BW], f16, tag="fvs")
                            nc.scalar.activation(fvs[:], pfv[:], Copy)
                            nc.vector.tensor_mul(
                                z0[:, 6 + c0:6 + c0 + BW], pf2[:], fvs[:])
                            pz = ppool.tile([CPT, BW], f32, tag="pz")
                            for k in range(K7):
                                nc.tensor.matmul(
                                    pz[:], lhsT(d7s, gt, K7, k),
                                    z0[:, c0 + k:c0 + k + BW],
                                    start=(k == 0), stop=(k == K7 - 1))
                            nc.vector.tensor_mul(
                                res[:, c0:c0 + BW], pz[:],
                                f1[:, c0:c0 + BW])

                        conv3s(0)
                        for t in range(1, NB):
                            conv3s(t)
                            zstage(t - 1)
                        zstage(NB - 1)
                        nc.sync.dma_start(outd[b, cs, :], res[:])

            def one_pass_pc():
                # PE: conv3(x2) + conv7. DVE: PAIR_MAC for fv/f1 taps 0-1 +
                # both gate muls (fp16 2x). Pool: tap-2 merges. ACT: evacs.
                for b in range(B):
                    for gt in range(NT):
                        cs = slice(gt * CPT, (gt + 1) * CPT)
                        xt1, xt2, xtv = load_unit(b, gt)
                        k0 = gt * K3

                        # fv taps 0,1 on DVE (one PAIR_MAC pass); tap 2 as an
                        # ACT scaled copy; Pool tensor_add merges them.
                        fv = mpool.tile([CPT, L], f16, tag="fv")
                        tv = mpool.tile([CPT, L], f16, tag="tv")
                        sv = mpool.tile([CPT, L], f16, tag="sv")
                        nc.vector._custom_dve(
                            pair_mac, out=tv[:],
                            in0=xtv[:, 0:L], in1=xtv[:, 1:1 + L],
                            s0=wvs[:, k0:k0 + 1], s1=wvs[:, k0 + 1:k0 + 2],
                            imm2=0.0)
                        nc.scalar.activation(
                            sv[:], xtv[:, 2:2 + L], Copy,
                            scale=wvs[:, k0 + 2:k0 + 3])
                        nc.vector.tensor_add(fv[:], tv[:], sv[:])

                        # f1 same split
                        f1 = mpool.tile([CPT, L], f16, tag="f1")
                        t1 = mpool.tile([CPT, L], f16, tag="t1")
                        s1 = mpool.tile([CPT, L], f16, tag="s1")
                        nc.vector._custom_dve(
                            pair_mac, out=t1[:],
                            in0=xt1[:, 0:L], in1=xt1[:, 1:1 + L],
                            s0=w1s[:, k0:k0 + 1], s1=w1s[:, k0 + 1:k0 + 2],
                            imm2=0.0)
                        nc.scalar.activation(
                            s1[:], xt1[:, 2:2 + L], Copy,
                            scale=w1s[:, k0 + 2:k0 + 3])
                        nc.vector.tensor_add(f1[:], t1[:], s1[:])

                        z0 = mpool.tile([CPT, 6 + L], f16, tag="z0")
                        nc.gpsimd.memset(z0[:, 0:6], 0.0)
                        res = opool.tile([CPT, L], f16, tag="res")
                        pf = {}
                        # "pcw": 1024-col double-bank stages halve the
                        # ACT<->DVE<->PE handoff count per unit
                        SW = 2 * BW if variant == "pcw" else BW
                        NS = L // SW

                        def conv3s(t):
                            c0 = t * SW
                            pf2 = ppool3.tile([CPT, SW], f32, tag="pf2")
                            for h in range(SW // BW):
                                hb = h * BW
                                for k in range(K3):
                                    nc.tensor.matmul(
                                        pf2[:, hb:hb + BW],
                                        lhsT(d2s, gt, K3, k),
                                        xt2[:, c0 + hb + k:c0 + hb + k + BW],
                                        start=(k == 0), stop=(k == K3 - 1))
                            pf[t] = pf2

                        def zstage(t):
                            c0 = t * SW
                            pf2 = pf.pop(t)
                            f2s = mpool.tile([CPT, SW], f16, tag="f2s")
                            nc.scalar.activation(f2s[:], pf2[:], Copy)
                            nc.vector.tensor_mul(
                                z0[:, 6 + c0:6 + c0 + SW], f2s[:],
                                fv[:, c0:c0 + SW])
                            pz = ppool.tile([CPT, SW], f32, tag="pz")
                            for h in range(SW // BW):
                                hb = h * BW
                                for k in range(K7):
                                    nc.tensor.matmul(
                                        pz[:, hb:hb + BW],
                                        lhsT(d7s, gt, K7, k),
                                        z0[:, c0 + hb + k:c0 + hb + k + BW],
                                        start=(k == 0), stop=(k == K7 - 1))
                            pzs = mpool.tile([CPT, SW], f16, tag="pzs")
                            nc.scalar.activation(pzs[:], pz[:], Copy)
                            nc.vector.tensor_mul(
                                res[:, c0:c0 + SW], pzs[:],
                                f1[:, c0:c0 + SW])

                        conv3s(0)
                        for t in range(1, NS):
                            conv3s(t)
                            zstage(t - 1)
                        zstage(NS - 1)
                        nc.sync.dma_start(outd[b, cs, :], res[:])

            body = {"pc": one_pass_pc, "pcq": one_pass_pc, "pcb": one_pass_pc,
                    "pcw": one_pass_pc}.get(variant, one_pass_v16)
            if hwloop and niter > 1:
                with tc.For_i(0, niter, 1):
                    body()
            else:
                for _ in range(niter):
                    body()

    nc.compile()
    return nc


def get_program(niter=1, variant=DEFAULT_VARIANT, hwloop=False):
    key = ("nc", niter, variant, hwloop)
    if key not in _PROG_CACHE:
        _PROG_CACHE[key] = build_program(niter, variant, hwloop)
    return _PROG_CACHE[key]


def _diag_blocks(w, K):
    """w: [DG, K] fp32 -> [CPT, NT*K*CPT] fp16 with
    out[p, (gt*K+k)*CPT + p] = w[gt*CPT + p, k]."""
    out = np.zeros((CPT, NT * K * CPT), np.float16)
    p = np.arange(CPT)
    for gt in range(NT):
        for k in range(K):
            out[p, (gt * K + k) * CPT + p] = w[gt * CPT:(gt + 1) * CPT,
                                               k].astype(np.float16)
    return out


def make_in_maps(x, w_proj, w_short):
    """Host-side sharding: de-interleave the 3 streams, cast to fp16, slice
    channels across cores; precompute per-channel tap weight tables."""
    x = np.asarray(x)
    w_proj = np.asarray(w_proj, dtype=np.float32)
    w_short = np.asarray(w_short, dtype=np.float32)
    # channel c = 3*g + p  ->  [B, G, 3, L]
    xr = x.reshape(B, D, 3, L).astype(np.float16)
    wp = w_proj[:, 0, :].reshape(D, 3, K3)
    w7_full = np.repeat(w_short[:, 0, :], D // w_short.shape[0], axis=0)
    in_maps = []
    for i in range(NCORES):
        g0, g1 = DG * i, DG * (i + 1)
        in_maps.append({
            "x1": np.ascontiguousarray(xr[:, g0:g1, 0, :]),
            "x2": np.ascontiguousarray(xr[:, g0:g1, 1, :]),
            "xv": np.ascontiguousarray(xr[:, g0:g1, 2, :]),
            "w1": np.ascontiguousarray(wp[g0:g1, 0, :]),
            "w2": np.ascontiguousarray(wp[g0:g1, 1, :]),
            "wv": np.ascontiguousarray(wp[g0:g1, 2, :]),
            "d2": _diag_blocks(wp[g0:g1, 1, :], K3),
            "dv": _diag_blocks(wp[g0:g1, 2, :], K3),
            "d7": _diag_blocks(w7_full[g0:g1, :], K7),
        })
    return in_maps


def kernel(x, w_proj, w_short):
    import os
    from concourse.bass_utils import run_bass_kernel_spmd

    nc = get_program(variant=DEFAULT_VARIANT)
    in_maps = make_in_maps(x, w_proj, w_short)
    try:
        res = run_bass_kernel_spmd(nc, in_maps, core_ids=list(range(NCORES)))
    except ModuleNotFoundError:
        # BASS_TRACE set but this axon client has no NTFF profile hook;
        # rerun with tracing off.
        os.environ["BASS_NEVER_TRACE"] = "1"
        res = run_bass_kernel_spmd(nc, in_maps, core_ids=list(range(NCORES)))
    out = np.concatenate([res.results[i]["out"] for i in range(NCORES)], axis=1)
    return np.ascontiguousarray(out.astype(np.float32))


# revision 33
# speedup vs baseline: 1.0091x; 1.0091x over previous
"""Trainium2 Bass kernel for nn_B2BConv1d (Hyena-style back-to-back causal
depthwise convs with gating).

Reference computation (B=2, D=4096, L=2048, channels of x are 3*D interleaved
as c = 3*g + p for stream p in {x1, x2, v}):
    features = causal_dw_conv1d(x, w_proj)          # K=3, per-channel weights
    x1, x2, v = de-interleave(features)             # [B, D, L] each
    z = x2 * v
    z = causal_dw_conv1d(z, repeat(w_short, 16))    # K=7, filter shared per 16ch
    out = x1 * z

Sharding: channels (g in [0, 4096)) split across 8 cores, 512 output channels
per core.  No halo needed (convs are along L, fully local per channel).
The host de-interleaves the 3 streams, casts them to fp16 (halves HBM read
traffic: 12 MiB/core instead of 24 MiB), and precomputes weight tables.

Engine plan per 128-channel x 2048 unit (variant "pc"):
  - TensorE: pf2 = diag-matmul conv3(x2) [3 taps] + pz = conv7(z) [7 taps]
    (10 PE taps instead of the baseline's 13).
  - ScalarE (ACT): tap-2 scaled copies for fv/f1, evacuates pf2/pz PSUM->fp16.
  - VectorE (DVE): PAIR_MAC custom op (2 conv taps in one pass via two
    shifted views) for fv and f1 taps 0-1; tap-2 merge adds and both gate
    multiplies as fp16 2x tensor_tensor.
  - Pool (gpsimd): pad memsets only (walrus rejects TensorScalarPtr on Pool;
    gpsimd tensor_add measured slower than the DVE 2x path).
  - DMA: all loads/stores HWDGE fp16.
Variant "v16" keeps the baseline 13-tap PE structure with fp16 loads.
"""

import numpy as np
from contextlib import ExitStack

B, D, L = 2, 4096, 2048
NCORES = 8
DG = D // NCORES          # 512 output channels per core
CPT = 128                 # channels per partition tile
NT = DG // CPT            # 4 partition tiles per core
K3, K7 = 3, 7
NB = 4                    # PSUM bank tiles per unit
BW = L // NB              # 512 columns per bank tile

import os as _os
DEFAULT_VARIANT = _os.environ.get("KVAR", "pc")

_PROG_CACHE = {}
_PAIR_MAC = None


def _pair_mac_uops_2x():
    """Hand-authored 2X_1PORT uop program for PAIR_MAC: each cycle processes
    an element pair (lo via SRC_0/SRC_1, hi via SRC_0_HI/SRC_1_HI), computing
    out = src0*c0 + src1*c1 for both halves.  out_lo is parked in delay lane 0
    at stage 3 and written via WR0_LO=DELAY_0; out_hi flows through the ALU
    bypass chain to WR0_HI=ALU_OUT."""
    from concourse.dve_uop import (
        AluInp, AluOp, DelayInp, InpSel, OutPath, OutSel, Trigger, UopConfig,
        UopDpConfig, DISABLE, ENABLE)

    HOLD = [DelayInp.PREV_DELAY] * 6 + [DelayInp.PREV_ALU_OUT]
    DEN = [1, 1, 1, 1, 1, 1, 0]

    def dp(op, a, b, cap=None):
        delay = list(HOLD)
        if cap is not None:
            delay[cap] = DelayInp.PREV_ALU_OUT
        return UopDpConfig(op=op, alu_src0=a, alu_src1=b, delay=delay,
                           alu_out_enable=1, delay_enable=list(DEN))

    M, A, BYP = AluOp.MULTIPLY, AluOp.ADD, AluOp.BYPASS
    P = AluInp
    stages = [
        dp(M, P.PREV_DELAY_0, P.PREV_DELAY_1),            # m0lo = s0lo*c0
        dp(M, P.PREV_DELAY_2, P.PREV_DELAY_3, cap=0),     # m1lo; lane0<-m0lo
        dp(A, P.PREV_DELAY_0, P.PREV_ALU_OUT),            # out_lo
        dp(M, P.PREV_DELAY_4, P.PREV_DELAY_1, cap=0),     # m0hi; lane0<-out_lo
        dp(M, P.PREV_DELAY_5, P.PREV_DELAY_3, cap=4),     # m1hi; lane4<-m0hi
        dp(A, P.PREV_DELAY_4, P.PREV_ALU_OUT),            # out_hi
        dp(BYP, P.PREV_ALU_OUT, P.PREV_ALU_OUT),
        dp(BYP, P.PREV_ALU_OUT, P.PREV_ALU_OUT),
    ]
    u = UopConfig(
        inp=[InpSel.ZERO, InpSel.SRC_0, InpSel.CONST_0, InpSel.SRC_1,
             InpSel.CONST_1, InpSel.SRC_0_HI, InpSel.SRC_1_HI, InpSel.ZERO],
        inp_enable=[0, 1, 1, 1, 1, 1, 1, 0],
        out={OutPath.WR0_LO: OutSel.DELAY_0, OutPath.WR0_HI: OutSel.ALU_OUT,
             OutPath.WR1_LO: OutSel.ALU_OUT, OutPath.WR1_HI: OutSel.ALU_OUT},
        out_enable={OutPath.WR0_LO: 1, OutPath.WR0_HI: 1,
                    OutPath.WR1_LO: 0, OutPath.WR1_HI: 0},
        require_inp0=1, require_inp1=1,
        trigger=(Trigger.SRC_TENSOR_DONE, Trigger.NONE, Trigger.NONE),
        datapath_config=stages,
    )
    u.validate("v3")
    return [u]


def _enable_pair_mac_2x(op):
    """Inject a compiled DveOpSpec with the 2x table variant into
    dve_ops._COMPILE_CACHE so dve_table_for_ops packs the 2X_1PORT slot."""
    import concourse.dve_ops as dve_ops
    from concourse.dve_spec import lower
    from concourse.dve_uop import DveOpSpec
    key = (op.name, "v3")
    cached = dve_ops._COMPILE_CACHE.get(key)
    if cached is not None and cached.uops_2x is not None:
        return
    spec2 = DveOpSpec(
        name=op.name, opcode=dve_ops.get_dve_sub_opcode(op.name),
        uops=lower(op.spec, ver="v3"), uops_2x=_pair_mac_uops_2x(),
        perf_max=1, rd1_en=True)
    dve_ops._COMPILE_CACHE[key] = spec2


def _get_pair_mac():
    """Register (once) and return the PAIR_MAC custom DVE op:
    out = in0*s0 + in1*s1 with per-partition scalars s0, s1 — two conv taps
    (two shifted views of the same SBUF tile) in one DVE pass."""
    global _PAIR_MAC
    if _PAIR_MAC is not None:
        return _PAIR_MAC
    import concourse.dve_ops as dve_ops
    from concourse.dve_spec import Spec, Src0, Src1, C0, C1
    from concourse.dve_uop import DveOpSpec
    from concourse.dve_spec import lower

    name = "PAIR_MAC_ANT"
    spec = Spec(
        body=Src0 * C0 + Src1 * C1,
        reference=lambda in0, in1, s0, s1, imm2: (
            in0.astype(np.float32) * s0 + in1.astype(np.float32) * s1),
    )
    # compute the uops sha for this arch so DveOp.compile's pin check passes
    shas = {}
    for ver in ("v3", "v4"):
        opcode = max(dve_ops._SUB_OPCODE_FOR_NAME.values()) + 1
        s = DveOpSpec(name=name, opcode=opcode, uops=lower(spec, ver=ver),
                      rd1_en=True)
        shas[ver] = s.sha(ver)
    op = dve_ops.DveOp(name, spec, subdim=False, uops_sha=shas)
    if name not in dve_ops._SUB_OPCODE_FOR_NAME:
        dve_ops.OPS.append(op)
        dve_ops._SUB_OPCODE_FOR_NAME[name] = (
            max(dve_ops._SUB_OPCODE_FOR_NAME.values()) + 1)
        dve_ops.CUSTOM_DVE_SPECS[name] = spec
    _PAIR_MAC = op
    return op


def build_program(niter=1, variant=DEFAULT_VARIANT, hwloop=False):
    """Build + compile the (SPMD, per-core) Bass program. Same program runs on
    all 8 cores; only the DRAM input contents differ.

    variants:
      "v16"   - fp16 loads, baseline engine split (PE does f2/fv conv3+conv7)
      "pc"    - fp16 loads, PE 10 taps, DVE pair-MACs + merges + gate muls
      "pcq"   - pc with loads spread over SP/ACT/DVE HWDGE queues
      "pcb"   - pc with 3-deep tile pools
      "pcw"   - pc with 1024-col double-bank pipeline stages
      "dma16" - loads + store only (DMA roofline probe)
    """
    import concourse.bacc as bacc
    import concourse.mybir as mybir
    import concourse.tile as tile

    f32 = mybir.dt.float32
    f16 = mybir.dt.float16
    mult = mybir.AluOpType.mult
    add = mybir.AluOpType.add
    Copy = mybir.ActivationFunctionType.Copy

    if variant in ("pc", "pcq", "pcb", "pcw", "pb", "p2"):
        pair_mac = _get_pair_mac()
        if variant == "p2":
            _enable_pair_mac_2x(pair_mac)

    nc = bacc.Bacc("TRN2", target_bir_lowering=False, debug=False)

    x1d = nc.dram_tensor("x1", [B, DG, L], f16, kind="ExternalInput")
    x2d = nc.dram_tensor("x2", [B, DG, L], f16, kind="ExternalInput")
    xvd = nc.dram_tensor("xv", [B, DG, L], f16, kind="ExternalInput")
    w1d = nc.dram_tensor("w1", [DG, K3], f32, kind="ExternalInput")
    w2d = nc.dram_tensor("w2", [DG, K3], f32, kind="ExternalInput")
    wvd = nc.dram_tensor("wv", [DG, K3], f32, kind="ExternalInput")
    d2d = nc.dram_tensor("d2", [CPT, NT * K3 * CPT], f16, kind="ExternalInput")
    dvd = nc.dram_tensor("dv", [CPT, NT * K3 * CPT], f16, kind="ExternalInput")
    d7d = nc.dram_tensor("d7", [CPT, NT * K7 * CPT], f16, kind="ExternalInput")
    outd = nc.dram_tensor("out", [B, DG, L], f16, kind="ExternalOutput")

    with tile.TileContext(nc) as tc:
        with ExitStack() as ctx:
            wpool = ctx.enter_context(tc.tile_pool(name="wpool", bufs=1))
            nb = 3 if variant == "pcb" else 2
            xpool = ctx.enter_context(tc.tile_pool(name="xpool", bufs=nb))
            mpool = ctx.enter_context(tc.tile_pool(name="mpool", bufs=nb))
            opool = ctx.enter_context(tc.tile_pool(name="opool", bufs=2))
            ppool = ctx.enter_context(
                tc.tile_pool(name="ppool", bufs=2, space="PSUM"))
            ppool3 = ctx.enter_context(
                tc.tile_pool(name="ppool3", bufs=2, space="PSUM"))

            # per-partition tap weights, one [CPT, K3] block per g-tile.
            # (Load only what the variant uses: pc/pcq has no fv/f2 stt or
            # dv-diag matmuls, so dvs/w2s stay unloaded.)
            w1s = wpool.tile([CPT, NT * K3], f32)
            wvs = wpool.tile([CPT, NT * K3], f32)
            for gt in range(NT):
                cs = slice(gt * CPT, (gt + 1) * CPT)
                nc.sync.dma_start(w1s[:, gt * K3:(gt + 1) * K3], w1d[cs, :])
                nc.sync.dma_start(wvs[:, gt * K3:(gt + 1) * K3], wvd[cs, :])
            # diag lhsT weight matrices for the PE convs
            d2s = wpool.tile([CPT, NT * K3 * CPT], f16)
            d7s = wpool.tile([CPT, NT * K7 * CPT], f16)
            nc.sync.dma_start(d2s[:], d2d[:, :])
            nc.sync.dma_start(d7s[:], d7d[:, :])
            if variant not in ("pc", "pcq", "pcb", "pcw", "pb", "p2"):
                dvs = wpool.tile([CPT, NT * K3 * CPT], f16)
                nc.sync.dma_start(dvs[:], dvd[:, :])

            def lhsT(dtile, gt, K, k):
                o = (gt * K + k) * CPT
                return dtile[:, o:o + CPT]

            def load_unit(b, gt):
                cs = slice(gt * CPT, (gt + 1) * CPT)
                xt1 = xpool.tile([CPT, 2 + L], f16, tag="xt1")
                xt2 = xpool.tile([CPT, 2 + L], f16, tag="xt2")
                xtv = xpool.tile([CPT, 2 + L], f16, tag="xtv")
                nc.gpsimd.memset(xt1[:, 0:2], 0.0)
                nc.gpsimd.memset(xt2[:, 0:2], 0.0)
                nc.gpsimd.memset(xtv[:, 0:2], 0.0)
                if variant == "pcq":
                    # spread the three stream loads across three HWDGE
                    # queues (SP / ACT / DVE) for DMA-engine parallelism
                    nc.sync.dma_start(xt1[:, 2:2 + L], x1d[b, cs, :])
                    nc.scalar.dma_start(xt2[:, 2:2 + L], x2d[b, cs, :])
                    nc.vector.dma_start(xtv[:, 2:2 + L], xvd[b, cs, :])
                else:
                    nc.sync.dma_start(xt1[:, 2:2 + L], x1d[b, cs, :])
                    nc.sync.dma_start(xt2[:, 2:2 + L], x2d[b, cs, :])
                    nc.sync.dma_start(xtv[:, 2:2 + L], xvd[b, cs, :])
                return xt1, xt2, xtv

            def one_pass_v16():
                # baseline engine split, fp16 loads. f1 accumulated fp32.
                for b in range(B):
                    for gt in range(NT):
                        cs = slice(gt * CPT, (gt + 1) * CPT)
                        xt1, xt2, xtv = load_unit(b, gt)
                        if variant == "dma16":
                            res = opool.tile([CPT, L], f16, tag="res")
                            nc.scalar.activation(res[:], xt1[:, 2:2 + L], Copy)
                            nc.sync.dma_start(outd[b, cs, :], res[:])
                            continue

                        # f1 path fp32: ACT tap0, DVE taps 1-2.
                        f1 = mpool.tile([CPT, L], f32, tag="f1")
                        nc.scalar.activation(
                            f1[:], xt1[:, 0:L], Copy,
                            scale=w1s[:, gt * K3:gt * K3 + 1])
                        for k in (1, 2):
                            nc.vector.scalar_tensor_tensor(
                                f1[:], xt1[:, k:k + L],
                                w1s[:, gt * K3 + k:gt * K3 + k + 1], f1[:],
                                mult, add)

                        z0 = mpool.tile([CPT, 6 + L], f16, tag="z0")
                        nc.gpsimd.memset(z0[:, 0:6], 0.0)
                        res = opool.tile([CPT, L], f16, tag="res")

                        # software-pipeline by one bank tile
                        pf = {}

                        def conv3s(t):
                            c0 = t * BW
                            pf2 = ppool3.tile([CPT, BW], f32, tag="pf2")
                            pfv = ppool3.tile([CPT, BW], f32, tag="pfv")
                            for k in range(K3):
                                nc.tensor.matmul(
                                    pfv[:], lhsT(dvs, gt, K3, k),
                                    xtv[:, c0 + k:c0 + k + BW],
                                    start=(k == 0), stop=(k == K3 - 1))
                            for k in range(K3):
                                nc.tensor.matmul(
                                    pf2[:], lhsT(d2s, gt, K3, k),
                                    xt2[:, c0 + k:c0 + k + BW],
                                    start=(k == 0), stop=(k == K3 - 1))
                            pf[t] = (pf2, pfv)

                        def zstage(t):
                            c0 = t * BW
                            pf2, pfv = pf.pop(t)
                            fvs = mpool.tile([CPT, BW], f16, tag="fvs")
                            nc.scalar.activation(fvs[:], pfv[:], Copy)
                            nc.vector.tensor_mul(
                                z0[:, 6 + c0:6 + c0 + BW], pf2[:], fvs[:])
                            pz = ppool.tile([CPT, BW], f32, tag="pz")
                            for k in range(K7):
                                nc.tensor.matmul(
                                    pz[:], lhsT(d7s, gt, K7, k),
                                    z0[:, c0 + k:c0 + k + BW],
                                    start=(k == 0), stop=(k == K7 - 1))
                            nc.vector.tensor_mul(
                                res[:, c0:c0 + BW], pz[:],
                                f1[:, c0:c0 + BW])

                        conv3s(0)
                        for t in range(1, NB):
                            conv3s(t)
                            zstage(t - 1)
                        zstage(NB - 1)
                        nc.sync.dma_start(outd[b, cs, :], res[:])

            def one_pass_pc():
                # PE: conv3(x2) + conv7. DVE: PAIR_MAC for fv/f1 taps 0-1 +
                # both gate muls (fp16 2x). Pool: tap-2 merges. ACT: evacs.
                for b in range(B):
                    for gt in range(NT):
                        cs = slice(gt * CPT, (gt + 1) * CPT)
                        xt1, xt2, xtv = load_unit(b, gt)
                        k0 = gt * K3

                        # fv taps 0,1 on DVE (one PAIR_MAC pass); tap 2 as an
                        # ACT scaled copy; Pool tensor_add merges them.
                        fv = mpool.tile([CPT, L], f16, tag="fv")
                        tv = mpool.tile([CPT, L], f16, tag="tv")
                        sv = mpool.tile([CPT, L], f16, tag="sv")
                        _i = nc.vector._custom_dve(
                            pair_mac, out=tv[:],
                            in0=xtv[:, 0:L], in1=xtv[:, 1:1 + L],
                            s0=wvs[:, k0:k0 + 1], s1=wvs[:, k0 + 1:k0 + 2],
                            imm2=0.0)
                        if variant == "p2":
                            _i.perf_max = 1
                        nc.scalar.activation(
                            sv[:], xtv[:, 2:2 + L], Copy,
                            scale=wvs[:, k0 + 2:k0 + 3])
                        nc.vector.tensor_add(fv[:], tv[:], sv[:])

                        # f1 same split
                        f1 = mpool.tile([CPT, L], f16, tag="f1")
                        t1 = mpool.tile([CPT, L], f16, tag="t1")
                        s1 = mpool.tile([CPT, L], f16, tag="s1")
                        _i = nc.vector._custom_dve(
                            pair_mac, out=t1[:],
                            in0=xt1[:, 0:L], in1=xt1[:, 1:1 + L],
                            s0=w1s[:, k0:k0 + 1], s1=w1s[:, k0 + 1:k0 + 2],
                            imm2=0.0)
                        if variant == "p2":
                            _i.perf_max = 1
                        nc.scalar.activation(
                            s1[:], xt1[:, 2:2 + L], Copy,
                            scale=w1s[:, k0 + 2:k0 + 3])
                        nc.vector.tensor_add(f1[:], t1[:], s1[:])

                        z0 = mpool.tile([CPT, 6 + L], f16, tag="z0")
                        nc.gpsimd.memset(z0[:, 0:6], 0.0)
                        res = opool.tile([CPT, L], f16, tag="res")
                        pf = {}
                        # "pcw": 1024-col double-bank stages halve the
                        # ACT<->DVE<->PE handoff count per unit
                        SW = 2 * BW if variant == "pcw" else BW
                        NS = L // SW

                        def conv3s(t):
                            c0 = t * SW
                            pf2 = ppool3.tile([CPT, SW], f32, tag="pf2")
                            for h in range(SW // BW):
                                hb = h * BW
                                for k in range(K3):
                                    nc.tensor.matmul(
                                        pf2[:, hb:hb + BW],
                                        lhsT(d2s, gt, K3, k),
                                        xt2[:, c0 + hb + k:c0 + hb + k + BW],
                                        start=(k == 0), stop=(k == K3 - 1))
                            pf[t] = pf2

                        def zstage(t):
                            c0 = t * SW
                            pf2 = pf.pop(t)
                            f2s = mpool.tile([CPT, SW], f16, tag="f2s")
                            nc.scalar.activation(f2s[:], pf2[:], Copy)
                            nc.vector.tensor_mul(
                                z0[:, 6 + c0:6 + c0 + SW], f2s[:],
                                fv[:, c0:c0 + SW])
                            pz = ppool.tile([CPT, SW], f32, tag="pz")
                            for h in range(SW // BW):
                                hb = h * BW
                                for k in range(K7):
                                    nc.tensor.matmul(
                                        pz[:, hb:hb + BW],
                                        lhsT(d7s, gt, K7, k),
                                        z0[:, c0 + hb + k:c0 + hb + k + BW],
                                        start=(k == 0), stop=(k == K7 - 1))
                            pzs = mpool.tile([CPT, SW], f16, tag="pzs")
                            nc.scalar.activation(pzs[:], pz[:], Copy)
                            nc.vector.tensor_mul(
                                res[:, c0:c0 + SW], pzs[:],
                                f1[:, c0:c0 + SW])

                        conv3s(0)
                        for t in range(1, NS):
                            conv3s(t)
                            zstage(t - 1)
                        zstage(NS - 1)
                        nc.sync.dma_start(outd[b, cs, :], res[:])

            def one_pass_pb():
                # batch-packed pc with flat [CPT, B*(2+L)] padded tiles: one
                # PAIR_MAC / scaled-copy / merge-add spans both batches (the
                # b1-boundary positions land in b1's pad columns, never
                # read), cutting DVE to 8 instructions per channel tile.
                XW = B * (2 + L)
                ZW = B * (6 + L)

                def bo(bb, off=2):
                    return bb * (2 + L) + off

                for gt in range(NT):
                    cs = slice(gt * CPT, (gt + 1) * CPT)
                    k0 = gt * K3
                    xt1 = xpool.tile([CPT, XW], f16, tag="xt1")
                    xt2 = xpool.tile([CPT, XW], f16, tag="xt2")
                    xtv = xpool.tile([CPT, XW], f16, tag="xtv")
                    for xt, xd in ((xt1, x1d), (xt2, x2d), (xtv, xvd)):
                        for bb in range(B):
                            nc.gpsimd.memset(xt[:, bo(bb, 0):bo(bb, 2)], 0.0)
                            nc.sync.dma_start(
                                xt[:, bo(bb):bo(bb) + L], xd[bb, cs, :])

                    fv = mpool.tile([CPT, XW], f16, tag="fv")
                    sv = mpool.tile([CPT, XW], f16, tag="sv")
                    nc.vector._custom_dve(
                        pair_mac, out=fv[:, 2:XW],
                        in0=xtv[:, 0:XW - 2], in1=xtv[:, 1:XW - 1],
                        s0=wvs[:, k0:k0 + 1], s1=wvs[:, k0 + 1:k0 + 2],
                        imm2=0.0)
                    nc.scalar.activation(
                        sv[:], xtv[:], Copy, scale=wvs[:, k0 + 2:k0 + 3])
                    nc.vector.tensor_add(fv[:, 2:XW], fv[:, 2:XW],
                                         sv[:, 2:XW])

                    f1 = mpool.tile([CPT, XW], f16, tag="f1")
                    s1 = mpool.tile([CPT, XW], f16, tag="s1")
                    nc.vector._custom_dve(
                        pair_mac, out=f1[:, 2:XW],
                        in0=xt1[:, 0:XW - 2], in1=xt1[:, 1:XW - 1],
                        s0=w1s[:, k0:k0 + 1], s1=w1s[:, k0 + 1:k0 + 2],
                        imm2=0.0)
                    nc.scalar.activation(
                        s1[:], xt1[:], Copy, scale=w1s[:, k0 + 2:k0 + 3])
                    nc.vector.tensor_add(f1[:, 2:XW], f1[:, 2:XW],
                                         s1[:, 2:XW])

                    z0 = mpool.tile([CPT, ZW], f16, tag="z0")
                    res = opool.tile([CPT, B * L], f16, tag="res")

                    for bb in range(B):
                        zb = bb * (6 + L)
                        nc.gpsimd.memset(z0[:, zb:zb + 6], 0.0)
                        f2s = mpool.tile([CPT, L], f16, tag="f2s")
                        for t in range(NB):
                            c0 = t * BW
                            pf2 = ppool3.tile([CPT, BW], f32, tag="pf2")
                            for k in range(K3):
                                nc.tensor.matmul(
                                    pf2[:], lhsT(d2s, gt, K3, k),
                                    xt2[:, bo(bb, 0) + c0 + k:
                                        bo(bb, 0) + c0 + k + BW],
                                    start=(k == 0), stop=(k == K3 - 1))
                            nc.scalar.activation(f2s[:, c0:c0 + BW], pf2[:],
                                                 Copy)
                        nc.vector.tensor_mul(
                            z0[:, zb + 6:zb + 6 + L], f2s[:],
                            fv[:, bo(bb):bo(bb) + L])
                        pzs = mpool.tile([CPT, L], f16, tag="pzs")
                        for t in range(NB):
                            c0 = t * BW
                            pz = ppool.tile([CPT, BW], f32, tag="pz")
                            for k in range(K7):
                                nc.tensor.matmul(
                                    pz[:], lhsT(d7s, gt, K7, k),
                                    z0[:, zb + c0 + k:zb + c0 + k + BW],
                                    start=(k == 0), stop=(k == K7 - 1))
                            nc.scalar.activation(pzs[:, c0:c0 + BW], pz[:],
                                                 Copy)
                        nc.vector.tensor_mul(
                            res[:, bb * L:(bb + 1) * L], pzs[:],
                            f1[:, bo(bb):bo(bb) + L])
                        nc.sync.dma_start(outd[bb, cs, :],
                                          res[:, bb * L:(bb + 1) * L])

            body = {"pc": one_pass_pc, "pcq": one_pass_pc, "pcb": one_pass_pc,
                    "pcw": one_pass_pc, "pb": one_pass_pb,
                    "p2": one_pass_pc}.get(
                variant, one_pass_v16)
            if hwloop and niter > 1:
                with tc.For_i(0, niter, 1):
                    body()
            else:
                for _ in range(niter):
                    body()

    if variant == "p2":
        # byte-36[7:6]: make the 2X_1PORT table slot engine-reachable
        for inst in nc.all_instructions():
            if type(inst).__name__ == "InstCustomDveAnt":
                inst.perf_max = 1
    nc.compile()
    return nc


def get_program(niter=1, variant=DEFAULT_VARIANT, hwloop=False):
    key = ("nc", niter, variant, hwloop)
    if key not in _PROG_CACHE:
        _PROG_CACHE[key] = build_program(niter, variant, hwloop)
    return _PROG_CACHE[key]


def _diag_blocks(w, K):
    """w: [DG, K] fp32 -> [CPT, NT*K*CPT] fp16 with
    out[p, (gt*K+k)*CPT + p] = w[gt*CPT + p, k]."""
    out = np.zeros((CPT, NT * K * CPT), np.float16)
    p = np.arange(CPT)
    for gt in range(NT):
        for k in range(K):
            out[p, (gt * K + k) * CPT + p] = w[gt * CPT:(gt + 1) * CPT,
                                               k].astype(np.float16)
    return out


def make_in_maps(x, w_proj, w_short):
    """Host-side sharding: de-interleave the 3 streams, cast to fp16, slice
    channels across cores; precompute per-channel tap weight tables."""
    x = np.asarray(x)
    w_proj = np.asarray(w_proj, dtype=np.float32)
    w_short = np.asarray(w_short, dtype=np.float32)
    # channel c = 3*g + p  ->  [B, G, 3, L]
    xr = x.reshape(B, D, 3, L).astype(np.float16)
    wp = w_proj[:, 0, :].reshape(D, 3, K3)
    w7_full = np.repeat(w_short[:, 0, :], D // w_short.shape[0], axis=0)
    in_maps = []
    for i in range(NCORES):
        g0, g1 = DG * i, DG * (i + 1)
        in_maps.append({
            "x1": np.ascontiguousarray(xr[:, g0:g1, 0, :]),
            "x2": np.ascontiguousarray(xr[:, g0:g1, 1, :]),
            "xv": np.ascontiguousarray(xr[:, g0:g1, 2, :]),
            "w1": np.ascontiguousarray(wp[g0:g1, 0, :]),
            "w2": np.ascontiguousarray(wp[g0:g1, 1, :]),
            "wv": np.ascontiguousarray(wp[g0:g1, 2, :]),
            "d2": _diag_blocks(wp[g0:g1, 1, :], K3),
            "dv": _diag_blocks(wp[g0:g1, 2, :], K3),
            "d7": _diag_blocks(w7_full[g0:g1, :], K7),
        })
    return in_maps


def kernel(x, w_proj, w_short):
    import os
    from concourse.bass_utils import run_bass_kernel_spmd

    nc = get_program(variant=DEFAULT_VARIANT)
    in_maps = make_in_maps(x, w_proj, w_short)
    try:
        res = run_bass_kernel_spmd(nc, in_maps, core_ids=list(range(NCORES)))
    except ModuleNotFoundError:
        # BASS_TRACE set but this axon client has no NTFF profile hook;
        # rerun with tracing off.
        os.environ["BASS_NEVER_TRACE"] = "1"
        res = run_bass_kernel_spmd(nc, in_maps, core_ids=list(range(NCORES)))
    out = np.concatenate([res.results[i]["out"] for i in range(NCORES)], axis=1)
    return np.ascontiguousarray(out.astype(np.float32))


# revision 35
# speedup vs baseline: 1.1412x; 1.1309x over previous
"""Trainium2 Bass kernel for nn_B2BConv1d (Hyena-style back-to-back causal
depthwise convs with gating).

Reference computation (B=2, D=4096, L=2048, channels of x are 3*D interleaved
as c = 3*g + p for stream p in {x1, x2, v}):
    features = causal_dw_conv1d(x, w_proj)          # K=3, per-channel weights
    x1, x2, v = de-interleave(features)             # [B, D, L] each
    z = x2 * v
    z = causal_dw_conv1d(z, repeat(w_short, 16))    # K=7, filter shared per 16ch
    out = x1 * z

Sharding: channels (g in [0, 4096)) split across 8 cores, 512 output channels
per core.  No halo needed (convs are along L, fully local per channel).
The host de-interleaves the 3 streams, casts them to fp16 (halves HBM read
traffic: 12 MiB/core instead of 24 MiB), and precomputes weight tables.

Engine plan per 128-channel x 2048 unit (variant "pc"):
  - TensorE: pf2 = diag-matmul conv3(x2) [3 taps] + pz = conv7(z) [7 taps]
    (10 PE taps instead of the baseline's 13).
  - ScalarE (ACT): tap-2 scaled copies for fv/f1, evacuates pf2/pz PSUM->fp16.
  - VectorE (DVE): PAIR_MAC custom op (2 conv taps in one pass via two
    shifted views) for fv and f1 taps 0-1; tap-2 merge adds and both gate
    multiplies as fp16 2x tensor_tensor.
  - Pool (gpsimd): pad memsets only (walrus rejects TensorScalarPtr on Pool;
    gpsimd tensor_add measured slower than the DVE 2x path).
  - DMA: all loads/stores HWDGE fp16.
Variant "v16" keeps the baseline 13-tap PE structure with fp16 loads.
"""

import numpy as np
from contextlib import ExitStack

B, D, L = 2, 4096, 2048
NCORES = 8
DG = D // NCORES          # 512 output channels per core
CPT = 128                 # channels per partition tile
NT = DG // CPT            # 4 partition tiles per core
K3, K7 = 3, 7
NB = 4                    # PSUM bank tiles per unit
BW = L // NB              # 512 columns per bank tile

import os as _os
DEFAULT_VARIANT = _os.environ.get("KVAR", "pc")

_PROG_CACHE = {}
_PAIR_MAC = None


def _pair_mac_uops_2x():
    """Hand-authored 2X_1PORT uop program for PAIR_MAC: each cycle processes
    an element pair (lo via SRC_0/SRC_1, hi via SRC_0_HI/SRC_1_HI), computing
    out = src0*c0 + src1*c1 for both halves.  out_lo is parked in delay lane 0
    at stage 3 and written via WR0_LO=DELAY_0; out_hi flows through the ALU
    bypass chain to WR0_HI=ALU_OUT."""
    from concourse.dve_uop import (
        AluInp, AluOp, DelayInp, InpSel, OutPath, OutSel, Trigger, UopConfig,
        UopDpConfig, DISABLE, ENABLE)

    HOLD = [DelayInp.PREV_DELAY] * 6 + [DelayInp.PREV_ALU_OUT]
    DEN = [1, 1, 1, 1, 1, 1, 0]

    def dp(op, a, b, cap=None):
        delay = list(HOLD)
        if cap is not None:
            delay[cap] = DelayInp.PREV_ALU_OUT
        return UopDpConfig(op=op, alu_src0=a, alu_src1=b, delay=delay,
                           alu_out_enable=1, delay_enable=list(DEN))

    M, A, BYP = AluOp.MULTIPLY, AluOp.ADD, AluOp.BYPASS
    P = AluInp
    stages = [
        dp(M, P.PREV_DELAY_0, P.PREV_DELAY_1),            # m0lo = s0lo*c0
        dp(M, P.PREV_DELAY_2, P.PREV_DELAY_3, cap=0),     # m1lo; lane0<-m0lo
        dp(A, P.PREV_DELAY_0, P.PREV_ALU_OUT),            # out_lo
        dp(M, P.PREV_DELAY_4, P.PREV_DELAY_1, cap=0),     # m0hi; lane0<-out_lo
        dp(M, P.PREV_DELAY_5, P.PREV_DELAY_3, cap=4),     # m1hi; lane4<-m0hi
        dp(A, P.PREV_DELAY_4, P.PREV_ALU_OUT),            # out_hi
        dp(BYP, P.PREV_ALU_OUT, P.PREV_ALU_OUT),
        dp(BYP, P.PREV_ALU_OUT, P.PREV_ALU_OUT),
    ]
    u = UopConfig(
        inp=[InpSel.ZERO, InpSel.SRC_0, InpSel.CONST_0, InpSel.SRC_1,
             InpSel.CONST_1, InpSel.SRC_0_HI, InpSel.SRC_1_HI, InpSel.ZERO],
        inp_enable=[0, 1, 1, 1, 1, 1, 1, 0],
        out={OutPath.WR0_LO: OutSel.DELAY_0, OutPath.WR0_HI: OutSel.ALU_OUT,
             OutPath.WR1_LO: OutSel.ALU_OUT, OutPath.WR1_HI: OutSel.ALU_OUT},
        out_enable={OutPath.WR0_LO: 1, OutPath.WR0_HI: 1,
                    OutPath.WR1_LO: 0, OutPath.WR1_HI: 0},
        require_inp0=1, require_inp1=1,
        trigger=(Trigger.SRC_TENSOR_DONE, Trigger.NONE, Trigger.NONE),
        datapath_config=stages,
    )
    u.validate("v3")
    return [u]


def _enable_pair_mac_2x(op):
    """Inject a compiled DveOpSpec with the 2x table variant into
    dve_ops._COMPILE_CACHE so dve_table_for_ops packs the 2X_1PORT slot."""
    import concourse.dve_ops as dve_ops
    from concourse.dve_spec import lower
    from concourse.dve_uop import DveOpSpec
    key = (op.name, "v3")
    cached = dve_ops._COMPILE_CACHE.get(key)
    if cached is not None and cached.uops_2x is not None:
        return
    spec2 = DveOpSpec(
        name=op.name, opcode=dve_ops.get_dve_sub_opcode(op.name),
        uops=lower(op.spec, ver="v3"), uops_2x=_pair_mac_uops_2x(),
        perf_max=1, rd1_en=True)
    dve_ops._COMPILE_CACHE[key] = spec2


def _get_pair_mac():
    """Register (once) and return the PAIR_MAC custom DVE op:
    out = in0*s0 + in1*s1 with per-partition scalars s0, s1 — two conv taps
    (two shifted views of the same SBUF tile) in one DVE pass."""
    global _PAIR_MAC
    if _PAIR_MAC is not None:
        return _PAIR_MAC
    import concourse.dve_ops as dve_ops
    from concourse.dve_spec import Spec, Src0, Src1, C0, C1
    from concourse.dve_uop import DveOpSpec
    from concourse.dve_spec import lower

    name = "PAIR_MAC_ANT"
    spec = Spec(
        body=Src0 * C0 + Src1 * C1,
        reference=lambda in0, in1, s0, s1, imm2: (
            in0.astype(np.float32) * s0 + in1.astype(np.float32) * s1),
    )
    # compute the uops sha for this arch so DveOp.compile's pin check passes
    shas = {}
    for ver in ("v3", "v4"):
        opcode = max(dve_ops._SUB_OPCODE_FOR_NAME.values()) + 1
        s = DveOpSpec(name=name, opcode=opcode, uops=lower(spec, ver=ver),
                      rd1_en=True)
        shas[ver] = s.sha(ver)
    op = dve_ops.DveOp(name, spec, subdim=False, uops_sha=shas)
    if name not in dve_ops._SUB_OPCODE_FOR_NAME:
        dve_ops.OPS.append(op)
        dve_ops._SUB_OPCODE_FOR_NAME[name] = (
            max(dve_ops._SUB_OPCODE_FOR_NAME.values()) + 1)
        dve_ops.CUSTOM_DVE_SPECS[name] = spec
    _PAIR_MAC = op
    return op


def build_program(niter=1, variant=DEFAULT_VARIANT, hwloop=False):
    """Build + compile the (SPMD, per-core) Bass program. Same program runs on
    all 8 cores; only the DRAM input contents differ.

    variants:
      "v16"   - fp16 loads, baseline engine split (PE does f2/fv conv3+conv7)
      "pc"    - fp16 loads, PE 10 taps, DVE pair-MACs + merges + gate muls
      "pcq"   - pc with loads spread over SP/ACT/DVE HWDGE queues
      "pcb"   - pc with 3-deep tile pools
      "pcw"   - pc with 1024-col double-bank pipeline stages
      "dma16" - loads + store only (DMA roofline probe)
    """
    import concourse.bacc as bacc
    import concourse.mybir as mybir
    import concourse.tile as tile

    f32 = mybir.dt.float32
    f16 = mybir.dt.float16
    mult = mybir.AluOpType.mult
    add = mybir.AluOpType.add
    Copy = mybir.ActivationFunctionType.Copy

    if variant in ("pc", "pcq", "pcb", "pcw", "pb", "p2", "p3"):
        pair_mac = _get_pair_mac()
        if variant == "p2":
            _enable_pair_mac_2x(pair_mac)

    nc = bacc.Bacc("TRN2", target_bir_lowering=False, debug=False)

    x1d = nc.dram_tensor("x1", [B, DG, L], f16, kind="ExternalInput")
    x2d = nc.dram_tensor("x2", [B, DG, L], f16, kind="ExternalInput")
    xvd = nc.dram_tensor("xv", [B, DG, L], f16, kind="ExternalInput")
    w1d = nc.dram_tensor("w1", [DG, K3], f32, kind="ExternalInput")
    w2d = nc.dram_tensor("w2", [DG, K3], f32, kind="ExternalInput")
    wvd = nc.dram_tensor("wv", [DG, K3], f32, kind="ExternalInput")
    d2d = nc.dram_tensor("d2", [CPT, NT * K3 * CPT], f16, kind="ExternalInput")
    dvd = nc.dram_tensor("dv", [CPT, NT * K3 * CPT], f16, kind="ExternalInput")
    d7d = nc.dram_tensor("d7", [CPT, NT * K7 * CPT], f16, kind="ExternalInput")
    outd = nc.dram_tensor("out", [B, DG, L], f16, kind="ExternalOutput")

    with tile.TileContext(nc) as tc:
        with ExitStack() as ctx:
            wpool = ctx.enter_context(tc.tile_pool(name="wpool", bufs=1))
            nb = 3 if variant == "pcb" else 2
            xpool = ctx.enter_context(tc.tile_pool(name="xpool", bufs=nb))
            mpool = ctx.enter_context(tc.tile_pool(name="mpool", bufs=nb))
            opool = ctx.enter_context(tc.tile_pool(name="opool", bufs=2))
            pnb = 3 if variant == "p3" else 2
            ppool = ctx.enter_context(
                tc.tile_pool(name="ppool", bufs=pnb, space="PSUM"))
            ppool3 = ctx.enter_context(
                tc.tile_pool(name="ppool3", bufs=pnb, space="PSUM"))

            # per-partition tap weights, one [CPT, K3] block per g-tile.
            # (Load only what the variant uses: pc/pcq has no fv/f2 stt or
            # dv-diag matmuls, so dvs/w2s stay unloaded.)
            w1s = wpool.tile([CPT, NT * K3], f32)
            wvs = wpool.tile([CPT, NT * K3], f32)
            for gt in range(NT):
                cs = slice(gt * CPT, (gt + 1) * CPT)
                nc.sync.dma_start(w1s[:, gt * K3:(gt + 1) * K3], w1d[cs, :])
                nc.sync.dma_start(wvs[:, gt * K3:(gt + 1) * K3], wvd[cs, :])
            # diag lhsT weight matrices for the PE convs
            d2s = wpool.tile([CPT, NT * K3 * CPT], f16)
            d7s = wpool.tile([CPT, NT * K7 * CPT], f16)
            nc.sync.dma_start(d2s[:], d2d[:, :])
            nc.sync.dma_start(d7s[:], d7d[:, :])
            if variant not in ("pc", "pcq", "pcb", "pcw", "pb", "p2", "p3"):
                dvs = wpool.tile([CPT, NT * K3 * CPT], f16)
                nc.sync.dma_start(dvs[:], dvd[:, :])

            def lhsT(dtile, gt, K, k):
                o = (gt * K + k) * CPT
                return dtile[:, o:o + CPT]

            def load_unit(b, gt):
                cs = slice(gt * CPT, (gt + 1) * CPT)
                xt1 = xpool.tile([CPT, 2 + L], f16, tag="xt1")
                xt2 = xpool.tile([CPT, 2 + L], f16, tag="xt2")
                xtv = xpool.tile([CPT, 2 + L], f16, tag="xtv")
                nc.gpsimd.memset(xt1[:, 0:2], 0.0)
                nc.gpsimd.memset(xt2[:, 0:2], 0.0)
                nc.gpsimd.memset(xtv[:, 0:2], 0.0)
                if variant == "pcq":
                    # split load issue between the SP and ACT HWDGE queues
                    # (DVE queue cannot initiate DMAs)
                    nc.sync.dma_start(xt1[:, 2:2 + L], x1d[b, cs, :])
                    nc.scalar.dma_start(xt2[:, 2:2 + L], x2d[b, cs, :])
                    nc.scalar.dma_start(xtv[:, 2:2 + L], xvd[b, cs, :])
                else:
                    nc.sync.dma_start(xt1[:, 2:2 + L], x1d[b, cs, :])
                    nc.sync.dma_start(xt2[:, 2:2 + L], x2d[b, cs, :])
                    nc.sync.dma_start(xtv[:, 2:2 + L], xvd[b, cs, :])
                return xt1, xt2, xtv

            def one_pass_v16():
                # baseline engine split, fp16 loads. f1 accumulated fp32.
                for b in range(B):
                    for gt in range(NT):
                        cs = slice(gt * CPT, (gt + 1) * CPT)
                        xt1, xt2, xtv = load_unit(b, gt)
                        if variant == "dma16":
                            res = opool.tile([CPT, L], f16, tag="res")
                            nc.scalar.activation(res[:], xt1[:, 2:2 + L], Copy)
                            nc.sync.dma_start(outd[b, cs, :], res[:])
                            continue

                        # f1 path fp32: ACT tap0, DVE taps 1-2.
                        f1 = mpool.tile([CPT, L], f32, tag="f1")
                        nc.scalar.activation(
                            f1[:], xt1[:, 0:L], Copy,
                            scale=w1s[:, gt * K3:gt * K3 + 1])
                        for k in (1, 2):
                            nc.vector.scalar_tensor_tensor(
                                f1[:], xt1[:, k:k + L],
                                w1s[:, gt * K3 + k:gt * K3 + k + 1], f1[:],
                                mult, add)

                        z0 = mpool.tile([CPT, 6 + L], f16, tag="z0")
                        nc.gpsimd.memset(z0[:, 0:6], 0.0)
                        res = opool.tile([CPT, L], f16, tag="res")

                        # software-pipeline by one bank tile
                        pf = {}

                        def conv3s(t):
                            c0 = t * BW
                            pf2 = ppool3.tile([CPT, BW], f32, tag="pf2")
                            pfv = ppool3.tile([CPT, BW], f32, tag="pfv")
                            for k in range(K3):
                                nc.tensor.matmul(
                                    pfv[:], lhsT(dvs, gt, K3, k),
                                    xtv[:, c0 + k:c0 + k + BW],
                                    start=(k == 0), stop=(k == K3 - 1))
                            for k in range(K3):
                                nc.tensor.matmul(
                                    pf2[:], lhsT(d2s, gt, K3, k),
                                    xt2[:, c0 + k:c0 + k + BW],
                                    start=(k == 0), stop=(k == K3 - 1))
                            pf[t] = (pf2, pfv)

                        def zstage(t):
                            c0 = t * BW
                            pf2, pfv = pf.pop(t)
                            fvs = mpool.tile([CPT, BW], f16, tag="fvs")
                            nc.scalar.activation(fvs[:], pfv[:], Copy)
                            nc.vector.tensor_mul(
                                z0[:, 6 + c0:6 + c0 + BW], pf2[:], fvs[:])
                            pz = ppool.tile([CPT, BW], f32, tag="pz")
                            for k in range(K7):
                                nc.tensor.matmul(
                                    pz[:], lhsT(d7s, gt, K7, k),
                                    z0[:, c0 + k:c0 + k + BW],
                                    start=(k == 0), stop=(k == K7 - 1))
                            nc.vector.tensor_mul(
                                res[:, c0:c0 + BW], pz[:],
                                f1[:, c0:c0 + BW])

                        conv3s(0)
                        for t in range(1, NB):
                            conv3s(t)
                            zstage(t - 1)
                        zstage(NB - 1)
                        nc.sync.dma_start(outd[b, cs, :], res[:])

            def one_pass_pc():
                # PE: conv3(x2) + conv7. DVE: PAIR_MAC for fv/f1 taps 0-1 +
                # both gate muls (fp16 2x). Pool: tap-2 merges. ACT: evacs.
                for b in range(B):
                    for gt in range(NT):
                        cs = slice(gt * CPT, (gt + 1) * CPT)
                        xt1, xt2, xtv = load_unit(b, gt)
                        k0 = gt * K3

                        # fv taps 0,1 on DVE (one PAIR_MAC pass); tap 2 as an
                        # ACT scaled copy; Pool tensor_add merges them.
                        fv = mpool.tile([CPT, L], f16, tag="fv")
                        tv = mpool.tile([CPT, L], f16, tag="tv")
                        sv = mpool.tile([CPT, L], f16, tag="sv")
                        _i = nc.vector._custom_dve(
                            pair_mac, out=tv[:],
                            in0=xtv[:, 0:L], in1=xtv[:, 1:1 + L],
                            s0=wvs[:, k0:k0 + 1], s1=wvs[:, k0 + 1:k0 + 2],
                            imm2=0.0)
                        if variant == "p2":
                            _i.perf_max = 1
                        nc.scalar.activation(
                            sv[:], xtv[:, 2:2 + L], Copy,
                            scale=wvs[:, k0 + 2:k0 + 3])
                        nc.vector.tensor_add(fv[:], tv[:], sv[:])

                        # f1 same split
                        f1 = mpool.tile([CPT, L], f16, tag="f1")
                        t1 = mpool.tile([CPT, L], f16, tag="t1")
                        s1 = mpool.tile([CPT, L], f16, tag="s1")
                        _i = nc.vector._custom_dve(
                            pair_mac, out=t1[:],
                            in0=xt1[:, 0:L], in1=xt1[:, 1:1 + L],
                            s0=w1s[:, k0:k0 + 1], s1=w1s[:, k0 + 1:k0 + 2],
                            imm2=0.0)
                        if variant == "p2":
                            _i.perf_max = 1
                        nc.scalar.activation(
                            s1[:], xt1[:, 2:2 + L], Copy,
                            scale=w1s[:, k0 + 2:k0 + 3])
                        nc.vector.tensor_add(f1[:], t1[:], s1[:])

                        z0 = mpool.tile([CPT, 6 + L], f16, tag="z0")
                        nc.gpsimd.memset(z0[:, 0:6], 0.0)
                        res = opool.tile([CPT, L], f16, tag="res")
                        pf = {}
                        # "pcw": 1024-col double-bank stages halve the
                        # ACT<->DVE<->PE handoff count per unit
                        SW = 2 * BW if variant == "pcw" else BW
                        NS = L // SW

                        def conv3s(t):
                            c0 = t * SW
                            pf2 = ppool3.tile([CPT, SW], f32, tag="pf2")
                            for h in range(SW // BW):
                                hb = h * BW
                                for k in range(K3):
                                    nc.tensor.matmul(
                                        pf2[:, hb:hb + BW],
                                        lhsT(d2s, gt, K3, k),
                                        xt2[:, c0 + hb + k:c0 + hb + k + BW],
                                        start=(k == 0), stop=(k == K3 - 1))
                            pf[t] = pf2

                        def zstage(t):
                            c0 = t * SW
                            pf2 = pf.pop(t)
                            f2s = mpool.tile([CPT, SW], f16, tag="f2s")
                            nc.scalar.activation(f2s[:], pf2[:], Copy)
                            nc.vector.tensor_mul(
                                z0[:, 6 + c0:6 + c0 + SW], f2s[:],
                                fv[:, c0:c0 + SW])
                            pz = ppool.tile([CPT, SW], f32, tag="pz")
                            for h in range(SW // BW):
                                hb = h * BW
                                for k in range(K7):
                                    nc.tensor.matmul(
                                        pz[:, hb:hb + BW],
                                        lhsT(d7s, gt, K7, k),
                                        z0[:, c0 + hb + k:c0 + hb + k + BW],
                                        start=(k == 0), stop=(k == K7 - 1))
                            pzs = mpool.tile([CPT, SW], f16, tag="pzs")
                            nc.scalar.activation(pzs[:], pz[:], Copy)
                            nc.vector.tensor_mul(
                                res[:, c0:c0 + SW], pzs[:],
                                f1[:, c0:c0 + SW])

                        conv3s(0)
                        for t in range(1, NS):
                            conv3s(t)
                            zstage(t - 1)
                        zstage(NS - 1)
                        nc.sync.dma_start(outd[b, cs, :], res[:])

            def one_pass_pb():
                # batch-packed pc with flat [CPT, B*(2+L)] padded tiles: one
                # PAIR_MAC / scaled-copy / merge-add spans both batches (the
                # b1-boundary positions land in b1's pad columns, never
                # read), cutting DVE to 8 instructions per channel tile.
                XW = B * (2 + L)
                ZW = B * (6 + L)

                def bo(bb, off=2):
                    return bb * (2 + L) + off

                for gt in range(NT):
                    cs = slice(gt * CPT, (gt + 1) * CPT)
                    k0 = gt * K3
                    xt1 = xpool.tile([CPT, XW], f16, tag="xt1")
                    xt2 = xpool.tile([CPT, XW], f16, tag="xt2")
                    xtv = xpool.tile([CPT, XW], f16, tag="xtv")
                    for xt, xd in ((xt1, x1d), (xt2, x2d), (xtv, xvd)):
                        for bb in range(B):
                            nc.gpsimd.memset(xt[:, bo(bb, 0):bo(bb, 2)], 0.0)
                            nc.sync.dma_start(
                                xt[:, bo(bb):bo(bb) + L], xd[bb, cs, :])

                    fv = mpool.tile([CPT, XW], f16, tag="fv")
                    sv = mpool.tile([CPT, XW], f16, tag="sv")
                    nc.vector._custom_dve(
                        pair_mac, out=fv[:, 2:XW],
                        in0=xtv[:, 0:XW - 2], in1=xtv[:, 1:XW - 1],
                        s0=wvs[:, k0:k0 + 1], s1=wvs[:, k0 + 1:k0 + 2],
                        imm2=0.0)
                    nc.scalar.activation(
                        sv[:], xtv[:], Copy, scale=wvs[:, k0 + 2:k0 + 3])
                    nc.vector.tensor_add(fv[:, 2:XW], fv[:, 2:XW],
                                         sv[:, 2:XW])

                    f1 = mpool.tile([CPT, XW], f16, tag="f1")
                    s1 = mpool.tile([CPT, XW], f16, tag="s1")
                    nc.vector._custom_dve(
                        pair_mac, out=f1[:, 2:XW],
                        in0=xt1[:, 0:XW - 2], in1=xt1[:, 1:XW - 1],
                        s0=w1s[:, k0:k0 + 1], s1=w1s[:, k0 + 1:k0 + 2],
                        imm2=0.0)
                    nc.scalar.activation(
                        s1[:], xt1[:], Copy, scale=w1s[:, k0 + 2:k0 + 3])
                    nc.vector.tensor_add(f1[:, 2:XW], f1[:, 2:XW],
                                         s1[:, 2:XW])

                    z0 = mpool.tile([CPT, ZW], f16, tag="z0")
                    res = opool.tile([CPT, B * L], f16, tag="res")

                    for bb in range(B):
                        zb = bb * (6 + L)
                        nc.gpsimd.memset(z0[:, zb:zb + 6], 0.0)
                        f2s = mpool.tile([CPT, L], f16, tag="f2s")
                        for t in range(NB):
                            c0 = t * BW
                            pf2 = ppool3.tile([CPT, BW], f32, tag="pf2")
                            for k in range(K3):
                                nc.tensor.matmul(
                                    pf2[:], lhsT(d2s, gt, K3, k),
                                    xt2[:, bo(bb, 0) + c0 + k:
                                        bo(bb, 0) + c0 + k + BW],
                                    start=(k == 0), stop=(k == K3 - 1))
                            nc.scalar.activation(f2s[:, c0:c0 + BW], pf2[:],
                                                 Copy)
                        nc.vector.tensor_mul(
                            z0[:, zb + 6:zb + 6 + L], f2s[:],
                            fv[:, bo(bb):bo(bb) + L])
                        pzs = mpool.tile([CPT, L], f16, tag="pzs")
                        for t in range(NB):
                            c0 = t * BW
                            pz = ppool.tile([CPT, BW], f32, tag="pz")
                            for k in range(K7):
                                nc.tensor.matmul(
                                    pz[:], lhsT(d7s, gt, K7, k),
                                    z0[:, zb + c0 + k:zb + c0 + k + BW],
                                    start=(k == 0), stop=(k == K7 - 1))
                            nc.scalar.activation(pzs[:, c0:c0 + BW], pz[:],
                                                 Copy)
                        nc.vector.tensor_mul(
                            res[:, bb * L:(bb + 1) * L], pzs[:],
                            f1[:, bo(bb):bo(bb) + L])
                        nc.sync.dma_start(outd[bb, cs, :],
                                          res[:, bb * L:(bb + 1) * L])

            body = {"pc": one_pass_pc, "pcq": one_pass_pc, "pcb": one_pass_pc,
                    "pcw": one_pass_pc, "pb": one_pass_pb,
                    "p2": one_pass_pc, "p3": one_pass_pc}.get(
                variant, one_pass_v16)
            if hwloop and niter > 1:
                with tc.For_i(0, niter, 1):
                    body()
            else:
                for _ in range(niter):
                    body()

    if variant == "p2":
        # byte-36[7:6]: make the 2X_1PORT table slot engine-reachable
        for inst in nc.all_instructions():
            if type(inst).__name__ == "InstCustomDveAnt":
                inst.perf_max = 1
    nc.compile()
    return nc


def get_program(niter=1, variant=DEFAULT_VARIANT, hwloop=False):
    key = ("nc", niter, variant, hwloop)
    if key not in _PROG_CACHE:
        _PROG_CACHE[key] = build_program(niter, variant, hwloop)
    return _PROG_CACHE[key]


def _diag_blocks(w, K):
    """w: [DG, K] fp32 -> [CPT, NT*K*CPT] fp16 with
    out[p, (gt*K+k)*CPT + p] = w[gt*CPT + p, k]."""
    out = np.zeros((CPT, NT * K * CPT), np.float16)
    p = np.arange(CPT)
    for gt in range(NT):
        for k in range(K):
            out[p, (gt * K + k) * CPT + p] = w[gt * CPT:(gt + 1) * CPT,
                                               k].astype(np.float16)
    return out


def make_in_maps(x, w_proj, w_short):
    """Host-side sharding: de-interleave the 3 streams, cast to fp16, slice
    channels across cores; precompute per-channel tap weight tables."""
    x = np.asarray(x)
    w_proj = np.asarray(w_proj, dtype=np.float32)
    w_short = np.asarray(w_short, dtype=np.float32)
    # channel c = 3*g + p  ->  [B, G, 3, L]
    xr = x.reshape(B, D, 3, L).astype(np.float16)
    wp = w_proj[:, 0, :].reshape(D, 3, K3)
    w7_full = np.repeat(w_short[:, 0, :], D // w_short.shape[0], axis=0)
    in_maps = []
    for i in range(NCORES):
        g0, g1 = DG * i, DG * (i + 1)
        in_maps.append({
            "x1": np.ascontiguousarray(xr[:, g0:g1, 0, :]),
            "x2": np.ascontiguousarray(xr[:, g0:g1, 1, :]),
            "xv": np.ascontiguousarray(xr[:, g0:g1, 2, :]),
            "w1": np.ascontiguousarray(wp[g0:g1, 0, :]),
            "w2": np.ascontiguousarray(wp[g0:g1, 1, :]),
            "wv": np.ascontiguousarray(wp[g0:g1, 2, :]),
            "d2": _diag_blocks(wp[g0:g1, 1, :], K3),
            "dv": _diag_blocks(wp[g0:g1, 2, :], K3),
            "d7": _diag_blocks(w7_full[g0:g1, :], K7),
        })
    return in_maps


def kernel(x, w_proj, w_short):
    import os
    from concourse.bass_utils import run_bass_kernel_spmd

    nc = get_program(variant=DEFAULT_VARIANT)
    in_maps = make_in_maps(x, w_proj, w_short)
    try:
        res = run_bass_kernel_spmd(nc, in_maps, core_ids=list(range(NCORES)))
    except ModuleNotFoundError:
        # BASS_TRACE set but this axon client has no NTFF profile hook;
        # rerun with tracing off.
        os.environ["BASS_NEVER_TRACE"] = "1"
        res = run_bass_kernel_spmd(nc, in_maps, core_ids=list(range(NCORES)))
    out = np.concatenate([res.results[i]["out"] for i in range(NCORES)], axis=1)
    return np.ascontiguousarray(out.astype(np.float32))
